# revision 25
# baseline (speedup 1.0000x reference)
"""Multi-head attention (N=4, L=2048, E=1024, H=16, DK=64) on 8 TRN2 cores.

The reference splits heads with a PLAIN RESHAPE (n, l, H*DK) -> (n, H, l, DK),
so "head" h is a contiguous block of 128 tokens whose 2048 attention positions
are (s, tok) pairs: p' = 128*s + tok, where s indexes sixteen 64-wide E-slices.
Per (batch, block):
    Qb = q[n, 128b:128b+128, :].reshape(2048, 64)   (same for K, V)
    out_block = softmax(Qb Kb^T / 8) Vb -> reshape(128, E) -> rows of out

Sharding: core c owns token rows [n, 256c : 256c+256) for every batch n (two
128-token blocks per batch).  Outputs are disjoint rows; the host scatters.
Zero inter-core communication.

Per-core layout (all matmuls bf16, fp32 PSUM):
  x_sb   [128 e_in, n, 8 a, 256 tok]      resident (host-prearranged DMA)
  q_sb   [128 = (sg, d), nn, B, g, 4, 128]  per batch-pair; 512-col F
         projections (both batches of a pair share each 8-matmul psum group).
  k_sb   -> k_dup [128, nn, 16 s, 256 tok] with every key tile replicated on
         BOTH partition halves, so any score matmul has lhsT/rhs on matching
         partitions.
  v_sb   [128 tok, nn, 2 B, 16 s, 65]     col 64 = ones (softmax denominator).
  Chunks c = (B, sg, g): 512 q' = {(2*a2+sg)*128+tok : a2 in [4g,4g+4)}.
  Attention phase (B, g): for each key tile j the two sg-chunks' score
  matmuls (K=64, partition bases 0/64) run ROW-PACKED concurrently; their
  PV matmuls share one V weight load.  exp over [128, 2, 512] PSUM is split
  by unit between ScalarE (true Exp, scale=1/8 folded) and the DVE
  (Schraudolph: bf16_bits = int16(x*(log2e*128/8) + (16256-C)) as ONE
  tensor_scalar through an int16 bitcast of the bf16 exps tile).
  The software pipeline advances j in PAIRS: scores for units k+1,k+2 are
  emitted back-to-back (adjacent row-packed pairs), then feed, then PV for
  units k-1,k.  PV accumulates [65, 512]; row 64 = denominator.
  Normalization is split per block B: as soon as block (n,B) finishes its
  last PV, its reciprocal + broadcast + multiply + out-projection pieces are
  queued, overlapping the next block's attention.
"""

import ml_dtypes
import numpy as np

import concourse.bass as bass
import concourse.mybir as mybir
import concourse.tile as tile
from concourse import bacc
from concourse.alu_op_type import AluOpType
from concourse.bass_utils import run_bass_kernel_spmd

N, L, E, H = 4, 2048, 1024, 16
DK = E // H  # 64
NC = 8
BPC = 2  # token blocks per core per batch
TPB = 128  # tokens per block
TPN = BPC * TPB  # 256 tokens per batch per core
TC = N * TPN  # 1024 tokens per core
P = 128
QC = 512  # q' chunk size
NS = 16  # s values (key tiles per block)
ET = E // P  # 8

F32 = mybir.dt.float32
BF16 = mybir.dt.bfloat16
I16 = mybir.dt.int16
MM_DT = BF16

# Schraudolph exp constants (bf16 bit domain), the 1/sqrt(DK)=1/8 folded in.
SCHR_A = 128.0 * 1.4426950408889634 / 8.0
SCHR_B = 127.0 * 128.0 - 5.5
# exp units (per key tile j) handled by the DVE instead of ScalarE, per B.
import os as _os
if _os.environ.get("NO_DVE_EXP"):
    DVE_JS = {0: (), 1: ()}
else:
    DVE_JS = {0: (2, 5, 9, 12, 14), 1: (3, 6, 10, 13)}


def build_nc():
    nc = bacc.Bacc("TRN2", target_bir_lowering=False, debug=False, num_devices=NC)

    # Host-prearranged layouts: per-partition-contiguous chunks so every DMA
    # descriptor is large (2-4KB) instead of 512B strided lines.
    #   xp[p, n, a, t]  = x[n, 256c+t, a*128+p]
    #   w*[p, c, a, d]  = W.T[a*128+p, c*128+d]
    xp = nc.dram_tensor("xp", [P, N, ET, TPN], MM_DT, kind="ExternalInput").ap()
    wqT = nc.dram_tensor("wqT", [P, ET, ET, P], MM_DT, kind="ExternalInput").ap()
    wkT = nc.dram_tensor("wkT", [P, ET, ET, P], MM_DT, kind="ExternalInput").ap()
    wvT = nc.dram_tensor("wvT", [P, ET, ET, P], MM_DT, kind="ExternalInput").ap()
    woT = nc.dram_tensor("woT", [P, ET, ET, P], MM_DT, kind="ExternalInput").ap()
    # output in bf16 (halves the out-DMA); the host casts back to f32.
    outp = nc.dram_tensor("outp", [TC, E], MM_DT, kind="ExternalOutput").ap()

    with tile.TileContext(nc) as tc:
        with (
            tc.tile_pool(name="const", bufs=1) as const,
            tc.tile_pool(name="wpool", bufs=1) as wpool,
            tc.tile_pool(name="xp", bufs=1) as x_pool,
            tc.tile_pool(name="qkv", bufs=2) as qkv_pool,
            tc.tile_pool(name="expp", bufs=4) as exp_pool,
            tc.tile_pool(name="opt", bufs=2) as opt_pool,
            tc.tile_pool(name="rcp", bufs=2) as rec_pool,
            tc.tile_pool(name="ops", bufs=2) as op_pool,
            tc.tile_pool(name="scps", bufs=2, space="PSUM") as sc_psum,
            tc.tile_pool(name="pvps", bufs=2, space="PSUM") as pv_psum,
            tc.tile_pool(name="misc", bufs=2, space="PSUM") as misc_psum,
        ):
            ones_f32 = const.tile([P, P], F32)
            nc.vector.memset(ones_f32[:], 1.0)
            ones_r = const.tile([P, P], mybir.dt.float32r)
            nc.vector.tensor_copy(ones_r[:], ones_f32[:])

            # ---- warmup: hold HAM at K=8/8 while input DMAs land ----
            warm_rhs = const.tile([P, QC], MM_DT)
            nc.vector.memset(warm_rhs[:], 0.001)
            for _w in range(8):
                wps = misc_psum.tile([P, QC], F32, tag="misc", name="warmps")
                nc.tensor.matmul(
                    wps[:, 0:TPN],
                    warm_rhs[:, 0:P],
                    warm_rhs[:, 0:TPN],
                    start=True,
                    stop=True,
                )

            # ---- resident weights + x; priority-ordered chunked DMAs ----
            engs = [nc.sync, nc.scalar, nc.gpsimd]

            def load_w_chunked(w_dram, nm, e0):
                w_sb = wpool.tile([P, ET, ET, P], MM_DT, tag=nm, name=nm)
                for c in range(ET):
                    eng = engs[(e0 + c) % len(engs)]
                    eng.dma_start(out=w_sb[:, c, :, :], in_=w_dram[:, c, :, :])
                return w_sb

            x_sb = x_pool.tile([P, N, ET, TPN], MM_DT, tag="x", name="x_sb")
            nc.sync.dma_start(out=x_sb[:, 0, :, :], in_=xp[:, 0, :, :])
            nc.scalar.dma_start(out=x_sb[:, 1, :, :], in_=xp[:, 1, :, :])
            wk_sb = load_w_chunked(wkT, "wk", 2)
            wq_sb = load_w_chunked(wqT, "wq", 0)
            wv_sb = load_w_chunked(wvT, "wv", 1)
            nc.gpsimd.dma_start(out=x_sb[:, 2, :, :], in_=xp[:, 2, :, :])
            nc.sync.dma_start(out=x_sb[:, 3, :, :], in_=xp[:, 3, :, :])
            wo_sb = load_w_chunked(woT, "wo", 0)

            # ---- per-PAIR projections (F=512 over both batches' tokens) ----
            def project_pair(pp):
                n0 = 2 * pp
                q_sb = qkv_pool.tile([P, 2, BPC, 2, 4, TPB], MM_DT, tag="q",
                                     name="q_sb")
                k_sb = qkv_pool.tile([P, 2, ET, TPN], MM_DT, tag="k",
                                     name="k_sb")
                k_dup = qkv_pool.tile([P, 2, NS, TPN], MM_DT, tag="kd",
                                      name="k_dup")
                v_sb = qkv_pool.tile([P, 2, BPC, NS, DK + 1], MM_DT, tag="v",
                                     name="v_sb")
                groups = []

                def qk_group(w_sb, dst, a2):
                    def emit():
                        ps = misc_psum.tile([P, QC], F32, tag="misc",
                                            name="qkps")
                        for a in range(ET):
                            nc.tensor.matmul(
                                ps[:],
                                w_sb[:, a2, a, :],
                                x_sb[:, n0 : n0 + 2, a, :],
                                start=(a == 0),
                                stop=(a == ET - 1),
                            )
                        if dst is q_sb:
                            nc.vector.tensor_copy(
                                dst[:, :, :, a2 // 4, a2 % 4, :],
                                ps.rearrange("p (nn b t) -> p nn b t",
                                             t=TPB, b=BPC),
                            )
                        else:
                            nc.vector.tensor_copy(
                                dst[:, :, a2, :],
                                ps.rearrange("p (nn t) -> p nn t", t=TPN),
                            )
                    return emit

                def kdup_group(nn):
                    def emit():
                        # replicate key tiles onto both partition halves;
                        # same-half copies (no partition shift) go to the
                        # otherwise-idle GpSimd, cross-half ones to the DVE.
                        nc.gpsimd.tensor_copy(k_dup[0:DK, nn, 0:NS:2, :],
                                              k_sb[0:DK, nn, :, :])
                        nc.gpsimd.tensor_copy(k_dup[DK:P, nn, 1:NS:2, :],
                                              k_sb[DK:P, nn, :, :])
                        nc.vector.tensor_copy(k_dup[DK:P, nn, 0:NS:2, :],
                                              k_sb[0:DK, nn, :, :])
                        nc.vector.tensor_copy(k_dup[0:DK, nn, 1:NS:2, :],
                                              k_sb[DK:P, nn, :, :])
                    return emit

                def v_group(nn, B, eh):
                    def emit():
                        ps = misc_psum.tile([P, QC], F32, tag="misc",
                                            name="vps")
                        for a in range(ET):
                            nc.tensor.matmul(
                                ps[:],
                                x_sb[:, n0 + nn, a, B * TPB : (B + 1) * TPB],
                                wv_sb[:, 4 * eh : 4 * eh + 4, a, :],
                                start=(a == 0),
                                stop=(a == ET - 1),
                            )
                        nc.vector.tensor_copy(
                            v_sb[:, nn, B, eh * (NS // 2)
                                 : (eh + 1) * (NS // 2), 0:DK],
                            ps.rearrange("p (s d) -> p s d", d=DK),
                        )
                    return emit

                def ones_group():
                    nc.vector.tensor_copy(
                        v_sb[:, :, :, :, DK].rearrange("p nn b s -> p (nn b s)"),
                        ones_f32[:, 0 : 2 * BPC * NS],
                    )

                for a2 in range(ET):
                    groups.append(qk_group(wk_sb, k_sb, a2))
                groups.append(kdup_group(0))
                groups.append(kdup_group(1))
                for a2 in range(ET):
                    groups.append(qk_group(wq_sb, q_sb, a2))
                for nn in range(2):
                    for B in range(BPC):
                        for eh in range(2):
                            groups.append(v_group(nn, B, eh))
                groups.append(ones_group)
                return (q_sb, k_dup, v_sb), groups

            def make_outproj_piece(opT, n, B, half):
                def emit():
                    ps = misc_psum.tile([P, QC], F32, tag="misc", name="opps")
                    for a2 in range(ET):
                        nc.tensor.matmul(
                            ps[:],
                            opT[:, a2, B, :],
                            wo_sb[:, 4 * half : 4 * half + 4, a2, :],
                            start=(a2 == 0),
                            stop=(a2 == ET - 1),
                        )
                    op_sb = op_pool.tile([P, QC], MM_DT, tag="op")
                    nc.vector.tensor_copy(op_sb[:], ps[:])
                    r0 = n * TPN + B * TPB
                    nc.sync.dma_start(
                        out=outp[r0 : r0 + TPB, half * QC : (half + 1) * QC],
                        in_=op_sb[:],
                    )
                return emit

            # ---- main pipeline (j-PAIRED software-pipelined emission) ----
            N_RUN = int(_os.environ.get("BATCHES", N))
            units = [
                (n, B, g, j)
                for n in range(N_RUN)
                for B in range(BPC)
                for g in range(2)
                for j in range(NS)
            ]
            NU = len(units)
            feed = []
            tiles_by_pair = {}
            ctx_by_batch = {}
            pv_by_phase = {}
            exps_by_unit = {}

            tiles0, groups0 = project_pair(0)
            tiles_by_pair[0] = tiles0
            for g_ in groups0:
                g_()

            def batch_start(n):
                if n % 2 == 0 and n // 2 + 1 < (N_RUN + 1) // 2:
                    nxt = project_pair(n // 2 + 1)
                    tiles_by_pair[n // 2 + 1] = nxt[0]
                    feed.extend(nxt[1])
                opT = opt_pool.tile([P, ET, BPC, TPB], MM_DT, tag="opT",
                                    name="opT")
                sums = [
                    rec_pool.tile([P, QC], F32, tag=f"sums{_i}",
                                  name=f"sums{_i}")
                    for _i in range(2)
                ]
                for _i in range(2):
                    nc.vector.memset(sums[_i][:], 1.0)
                ctx_by_batch[n] = (opT, sums)

            def make_norm_group(opT, rec, B):
                # the four 1/denom broadcasts of block (n,B) batched in one
                # feed item: pairs of row-packed concurrent matmuls (distinct
                # 32-row groups), then DVE multiplies normalize opT in place.
                def emit():
                    for half in range(2):
                        bcps = []
                        for sg_ in range(2):
                            c = B * 4 + sg_ * 2 + half
                            rp_ = 32 * (c % 4)
                            bcp = misc_psum.tile([P, QC], F32, tag="misc",
                                                 name="bcp")
                            bcps.append(bcp)
                            nc.tensor.matmul(
                                bcp[:],
                                ones_r[rp_ : rp_ + 1, :],
                                rec[rp_ : rp_ + 1, :],
                                start=True,
                                stop=True,
                                tile_position=(rp_, 0),
                            )
                        for sg_ in range(2):
                            g_ = half
                            dst = opT[sg_ * DK : (sg_ + 1) * DK,
                                      4 * g_ : 4 * g_ + 4, B, :]
                            nc.vector.tensor_mul(
                                dst,
                                dst,
                                bcps[sg_][sg_ * DK : (sg_ + 1) * DK, :]
                                .rearrange("d (a t) -> d a t", t=TPB),
                            )
                return emit

            def block_end(n, B):
                # as soon as block (n,B)'s PVs finish: reciprocal of its
                # denominators, then queue its norm + out-projection pieces.
                opT, sums = ctx_by_batch[n]
                rec = rec_pool.tile([P, QC], mybir.dt.float32r,
                                    tag=f"rec{B}", name=f"rec{B}")
                rsc = rec_pool.tile([P, QC], F32, tag=f"rsc{B}",
                                    name=f"rsc{B}")
                with nc.allow_low_precision(reason="softmax denominators"):
                    nc.vector.reciprocal_approx_fast(rsc[:], sums[B][:])
                    nc.vector.tensor_copy(rec[:], rsc[:])
                feed.append(make_norm_group(opT, rec, B))
                for half in range(2):
                    feed.append(make_outproj_piece(opT, n, B, half))

            def emit_scores_exp(k):
                n, B, g, j = units[k]
                if j == 0 and B == 0 and g == 0:
                    batch_start(n)
                q_sb, k_dup, v_sb = tiles_by_pair[n // 2]
                nn = n % 2
                if j == 0:
                    pv_by_phase[(n, B, g)] = [
                        pv_psum.tile([DK + 1, QC], F32, tag="pv",
                                     name=f"pv{_s}")
                        for _s in range(2)
                    ]
                tsl = slice(B * TPB, (B + 1) * TPB)
                sc = sc_psum.tile([P, 2, QC], F32, tag="sc", name="sc")
                exps = exp_pool.tile([P, 2 * QC], MM_DT, tag="exps",
                                     name="exps")
                exps_by_unit[k] = exps
                for sg in range(2):
                    dsl = slice(sg * DK, (sg + 1) * DK)
                    nc.tensor.matmul(
                        sc[:, sg, :],
                        k_dup[dsl, nn, j, tsl],
                        q_sb[dsl, nn, B, g, :, :],
                        start=True,
                        stop=True,
                    )
                sc_flat = sc.rearrange("p s q -> p (s q)")
                if j in DVE_JS[B]:
                    with nc.allow_low_precision(reason="schraudolph exp"):
                        nc.vector.tensor_scalar(
                            exps[:].bitcast(I16),
                            sc_flat,
                            SCHR_A,
                            SCHR_B,
                            AluOpType.mult,
                            AluOpType.add,
                        )
                else:
                    nc.scalar.activation(
                        exps[:],
                        sc_flat,
                        mybir.ActivationFunctionType.Exp,
                        scale=1.0 / np.sqrt(DK),
                    )

            def emit_pv_and_finish(k):
                n, B, g, j = units[k]
                q_sb, k_dup, v_sb = tiles_by_pair[n // 2]
                nn = n % 2
                pv = pv_by_phase[(n, B, g)]
                exps = exps_by_unit.pop(k)
                for sg in range(2):
                    nc.tensor.matmul(
                        pv[sg][:],
                        v_sb[:, nn, B, j, :],
                        exps[:, sg * QC : (sg + 1) * QC],
                        start=(j == 0),
                        stop=(j == NS - 1),
                    )
                if j == NS - 1:
                    opT, sums = ctx_by_batch[n]
                    qa = slice(4 * g, 4 * g + 4)
                    for sg in range(2):
                        c = B * 4 + sg * 2 + g
                        rp = 32 * (c % 4)
                        nc.vector.tensor_copy(
                            sums[c // 4][rp : rp + 1, :],
                            pv[sg][DK : DK + 1, :],
                        )
                        # split the two opT evictions across ScalarE and the
                        # DVE so the pv banks recycle ~2x sooner (the next
                        # phase's first PV has a WAR on them).
                        dst = opT[sg * DK : (sg + 1) * DK, qa, B, :]
                        src = pv[sg][0:DK, :].rearrange(
                            "d (a t) -> d a t", t=TPB
                        )
                        if sg == 0:
                            nc.scalar.activation(
                                dst, src, mybir.ActivationFunctionType.Copy
                            )
                        else:
                            nc.vector.tensor_copy(dst, src)
                    del pv_by_phase[(n, B, g)]
                    if g == 1:
                        block_end(n, B)

            # scores run two units ahead, emitted in adjacent PAIRS so the
            # two row-packed score matmul pairs sit back-to-back on the PE.
            emit_scores_exp(0)
            emit_scores_exp(1)
            NP2 = NU // 2
            for m in range(NP2 + 1):
                if m + 1 <= NP2 - 1:
                    emit_scores_exp(2 * m + 2)
                    emit_scores_exp(2 * m + 3)
                if feed:
                    rem = 32 - (m % 32)
                    take = (len(feed) + rem - 1) // rem
                    for _ in range(min(take, len(feed))):
                        feed.pop(0)()
                if m < NP2:
                    emit_pv_and_finish(2 * m)
                    emit_pv_and_finish(2 * m + 1)

            while feed:
                feed.pop(0)()

    nc.compile()
    return nc


_CACHED_NC = None


def get_nc():
    global _CACHED_NC
    if _CACHED_NC is None:
        _CACHED_NC = build_nc()
    return _CACHED_NC


def make_in_maps(inputs):
    x = np.ascontiguousarray(np.asarray(inputs["x"], dtype=np.float32))
    Wq = np.asarray(inputs["Wq"], dtype=np.float32)
    Wk = np.asarray(inputs["Wk"], dtype=np.float32)
    Wv = np.asarray(inputs["Wv"], dtype=np.float32)
    Wo = np.asarray(inputs["Wo"], dtype=np.float32)

    def cast(a):
        return np.ascontiguousarray(a).astype(ml_dtypes.bfloat16)

    def prew(W):
        # w_pre[p, c, a, d] = W.T[a*128+p, c*128+d]
        wT = W.T.reshape(ET, P, ET, P)
        return cast(wT.transpose(1, 2, 0, 3))

    wqT = prew(Wq)
    wkT = prew(Wk)
    wvT = prew(Wv)
    woT = prew(Wo)
    xr = x.reshape(N, L, E)

    in_maps = []
    for c in range(NC):
        xc = xr[:, 256 * c : 256 * (c + 1), :]  # [N, 256, E]
        # xp[p, n, a, t] = xc[n, t, a*128+p]
        xpc = xc.transpose(2, 0, 1).reshape(ET, P, N, TPN).transpose(1, 2, 0, 3)
        in_maps.append(
            {
                "xp": cast(xpc),
                "wqT": wqT,
                "wkT": wkT,
                "wvT": wvT,
                "woT": woT,
            }
        )
    return in_maps


def kernel(x, Wq, Wk, Wv, Wo):
    in_maps = make_in_maps({"x": x, "Wq": Wq, "Wk": Wk, "Wv": Wv, "Wo": Wo})
    res = run_bass_kernel_spmd(get_nc(), in_maps, list(range(NC)))
    out = np.empty((N, L, E), dtype=np.float32)
    for c in range(NC):
        o = res.results[c]["outp"].astype(np.float32).reshape(N, TPN, E)
        out[:, 256 * c : 256 * (c + 1), :] = o
    return out


# revision 26
# speedup vs baseline: 1.1028x; 1.1028x over previous
"""Multi-head attention (N=4, L=2048, E=1024, H=16, DK=64) on 8 TRN2 cores.

The reference splits heads with a PLAIN RESHAPE (n, l, H*DK) -> (n, H, l, DK),
so "head" h is a contiguous block of 128 tokens whose 2048 attention positions
are (s, tok) pairs: p' = 128*s + tok, where s indexes sixteen 64-wide E-slices.
Per (batch, block):
    Qb = q[n, 128b:128b+128, :].reshape(2048, 64)   (same for K, V)
    out_block = softmax(Qb Kb^T / 8) Vb -> reshape(128, E) -> rows of out

Sharding: core c owns token rows [n, 256c : 256c+256) for every batch n (two
128-token blocks per batch).  Outputs are disjoint rows; the host scatters.
Zero inter-core communication.

Per-core layout (all matmuls bf16, fp32 PSUM):
  x_sb   [128 e_in, n, 8 a, 256 tok]      resident (host-prearranged DMA)
  q_sb   [128 = (sg, d), nn, B, g, 4, 128]  per batch-pair; 512-col F
         projections (both batches of a pair share each 8-matmul psum group).
  k_sb   -> k_dup [128, nn, 16 s, 256 tok] with every key tile replicated on
         BOTH partition halves, so any score matmul has lhsT/rhs on matching
         partitions.
  v_sb   [128 tok, nn, 2 B, 16 s, 65]     col 64 = ones (softmax denominator).
  Chunks c = (B, sg, g): 512 q' = {(2*a2+sg)*128+tok : a2 in [4g,4g+4)}.
  Attention phase (B, g): for each key tile j the two sg-chunks' score
  matmuls (K=64, partition bases 0/64) run ROW-PACKED concurrently; their
  PV matmuls share one V weight load.  exp over [128, 2, 512] PSUM is split
  by unit between ScalarE (true Exp, scale=1/8 folded) and the DVE
  (Schraudolph: bf16_bits = int16(x*(log2e*128/8) + (16256-C)) as ONE
  tensor_scalar through an int16 bitcast of the bf16 exps tile).
  The software pipeline advances j in PAIRS: scores for units k+1,k+2 are
  emitted back-to-back (adjacent row-packed pairs), then feed, then PV for
  units k-1,k.  PV accumulates [65, 512]; row 64 = denominator.
  Normalization is split per block B: as soon as block (n,B) finishes its
  last PV, its reciprocal + broadcast + multiply + out-projection pieces are
  queued, overlapping the next block's attention.
"""

import ml_dtypes
import numpy as np

import concourse.bass as bass
import concourse.mybir as mybir
import concourse.tile as tile
from concourse import bacc
from concourse.alu_op_type import AluOpType
from concourse.bass_utils import run_bass_kernel_spmd

N, L, E, H = 4, 2048, 1024, 16
DK = E // H  # 64
NC = 8
BPC = 2  # token blocks per core per batch
TPB = 128  # tokens per block
TPN = BPC * TPB  # 256 tokens per batch per core
TC = N * TPN  # 1024 tokens per core
P = 128
QC = 512  # q' chunk size
NS = 16  # s values (key tiles per block)
ET = E // P  # 8

F32 = mybir.dt.float32
BF16 = mybir.dt.bfloat16
I16 = mybir.dt.int16
MM_DT = BF16

# Schraudolph exp constants (bf16 bit domain), the 1/sqrt(DK)=1/8 folded in.
SCHR_A = 128.0 * 1.4426950408889634 / 8.0
SCHR_B = 127.0 * 128.0 - 5.5
# exp units (per key tile j) handled by the DVE instead of ScalarE, per B.
import os as _os
if _os.environ.get("NO_DVE_EXP"):
    DVE_JS = {0: (), 1: ()}
else:
    DVE_JS = {0: (2, 5, 9, 12, 14), 1: (3, 6, 10, 13)}


def build_nc():
    nc = bacc.Bacc("TRN2", target_bir_lowering=False, debug=False, num_devices=NC)

    # Host-prearranged layouts: per-partition-contiguous chunks so every DMA
    # descriptor is large (2-4KB) instead of 512B strided lines.
    #   xp[p, n, a, t]  = x[n, 256c+t, a*128+p]
    #   w*[p, c, a, d]  = W.T[a*128+p, c*128+d]
    xp = nc.dram_tensor("xp", [P, N, ET, TPN], MM_DT, kind="ExternalInput").ap()
    wqT = nc.dram_tensor("wqT", [P, ET, ET, P], MM_DT, kind="ExternalInput").ap()
    wkT = nc.dram_tensor("wkT", [P, ET, ET, P], MM_DT, kind="ExternalInput").ap()
    wvT = nc.dram_tensor("wvT", [P, ET, ET, P], MM_DT, kind="ExternalInput").ap()
    woT = nc.dram_tensor("woT", [P, ET, ET, P], MM_DT, kind="ExternalInput").ap()
    # output in bf16 (halves the out-DMA); the host casts back to f32.
    outp = nc.dram_tensor("outp", [TC, E], MM_DT, kind="ExternalOutput").ap()

    with tile.TileContext(nc) as tc:
        with (
            tc.tile_pool(name="const", bufs=1) as const,
            tc.tile_pool(name="wpool", bufs=1) as wpool,
            tc.tile_pool(name="xp", bufs=1) as x_pool,
            tc.tile_pool(name="qkv", bufs=2) as qkv_pool,
            tc.tile_pool(name="expp", bufs=4) as exp_pool,
            tc.tile_pool(name="opt", bufs=2) as opt_pool,
            tc.tile_pool(name="rcp", bufs=2) as rec_pool,
            tc.tile_pool(name="ops", bufs=2) as op_pool,
            tc.tile_pool(name="scps", bufs=2, space="PSUM") as sc_psum,
            tc.tile_pool(name="pvps", bufs=2, space="PSUM") as pv_psum,
            tc.tile_pool(name="misc", bufs=2, space="PSUM") as misc_psum,
        ):
            ones_f32 = const.tile([P, P], F32)
            nc.vector.memset(ones_f32[:], 1.0)
            ones_r = const.tile([P, P], mybir.dt.float32r)
            nc.vector.tensor_copy(ones_r[:], ones_f32[:])

            # ---- warmup: hold HAM at K=8/8 while input DMAs land ----
            warm_rhs = const.tile([P, QC], MM_DT)
            nc.vector.memset(warm_rhs[:], 0.001)
            for _w in range(8):
                wps = misc_psum.tile([P, QC], F32, tag="misc", name="warmps")
                nc.tensor.matmul(
                    wps[:, 0:TPN],
                    warm_rhs[:, 0:P],
                    warm_rhs[:, 0:TPN],
                    start=True,
                    stop=True,
                )

            # ---- resident weights + x; priority-ordered chunked DMAs ----
            engs = [nc.sync, nc.scalar, nc.gpsimd]

            def load_w_chunked(w_dram, nm, e0):
                w_sb = wpool.tile([P, ET, ET, P], MM_DT, tag=nm, name=nm)
                for c in range(ET):
                    eng = engs[(e0 + c) % len(engs)]
                    eng.dma_start(out=w_sb[:, c, :, :], in_=w_dram[:, c, :, :])
                return w_sb

            x_sb = x_pool.tile([P, N, ET, TPN], MM_DT, tag="x", name="x_sb")
            nc.sync.dma_start(out=x_sb[:, 0, :, :], in_=xp[:, 0, :, :])
            nc.scalar.dma_start(out=x_sb[:, 1, :, :], in_=xp[:, 1, :, :])
            wk_sb = load_w_chunked(wkT, "wk", 2)
            wq_sb = load_w_chunked(wqT, "wq", 0)
            wv_sb = load_w_chunked(wvT, "wv", 1)
            nc.gpsimd.dma_start(out=x_sb[:, 2, :, :], in_=xp[:, 2, :, :])
            nc.sync.dma_start(out=x_sb[:, 3, :, :], in_=xp[:, 3, :, :])
            wo_sb = load_w_chunked(woT, "wo", 0)

            # ---- per-PAIR projections (F=512 over both batches' tokens) ----
            def project_pair(pp):
                n0 = 2 * pp
                q_sb = qkv_pool.tile([P, 2, BPC, 2, 4, TPB], MM_DT, tag="q",
                                     name="q_sb")
                k_sb = qkv_pool.tile([P, 2, ET, TPN], MM_DT, tag="k",
                                     name="k_sb")
                k_dup = qkv_pool.tile([P, 2, NS, TPN], MM_DT, tag="kd",
                                      name="k_dup")
                v_sb = qkv_pool.tile([P, 2, BPC, NS, DK + 1], MM_DT, tag="v",
                                     name="v_sb")
                groups = []

                def qk_group(w_sb, dst, a2):
                    def emit():
                        ps = misc_psum.tile([P, QC], F32, tag="misc",
                                            name="qkps")
                        for a in range(ET):
                            nc.tensor.matmul(
                                ps[:],
                                w_sb[:, a2, a, :],
                                x_sb[:, n0 : n0 + 2, a, :],
                                start=(a == 0),
                                stop=(a == ET - 1),
                            )
                        if dst is q_sb:
                            nc.vector.tensor_copy(
                                dst[:, :, :, a2 // 4, a2 % 4, :],
                                ps.rearrange("p (nn b t) -> p nn b t",
                                             t=TPB, b=BPC),
                            )
                        else:
                            nc.vector.tensor_copy(
                                dst[:, :, a2, :],
                                ps.rearrange("p (nn t) -> p nn t", t=TPN),
                            )
                    return emit

                def kdup_group(nn):
                    def emit():
                        # replicate every key tile onto both partition halves
                        for h in range(2):
                            hs = slice(h * DK, (h + 1) * DK)
                            nc.vector.tensor_copy(k_dup[hs, nn, 0:NS:2, :],
                                                  k_sb[0:DK, nn, :, :])
                            nc.vector.tensor_copy(k_dup[hs, nn, 1:NS:2, :],
                                                  k_sb[DK:P, nn, :, :])
                    return emit

                def v_group(nn, B, eh):
                    def emit():
                        ps = misc_psum.tile([P, QC], F32, tag="misc",
                                            name="vps")
                        for a in range(ET):
                            nc.tensor.matmul(
                                ps[:],
                                x_sb[:, n0 + nn, a, B * TPB : (B + 1) * TPB],
                                wv_sb[:, 4 * eh : 4 * eh + 4, a, :],
                                start=(a == 0),
                                stop=(a == ET - 1),
                            )
                        nc.vector.tensor_copy(
                            v_sb[:, nn, B, eh * (NS // 2)
                                 : (eh + 1) * (NS // 2), 0:DK],
                            ps.rearrange("p (s d) -> p s d", d=DK),
                        )
                    return emit

                def ones_group():
                    nc.vector.tensor_copy(
                        v_sb[:, :, :, :, DK].rearrange("p nn b s -> p (nn b s)"),
                        ones_f32[:, 0 : 2 * BPC * NS],
                    )

                for a2 in range(ET):
                    groups.append(qk_group(wk_sb, k_sb, a2))
                groups.append(kdup_group(0))
                groups.append(kdup_group(1))
                for a2 in range(ET):
                    groups.append(qk_group(wq_sb, q_sb, a2))
                for nn in range(2):
                    for B in range(BPC):
                        for eh in range(2):
                            groups.append(v_group(nn, B, eh))
                groups.append(ones_group)
                return (q_sb, k_dup, v_sb), groups

            def make_outproj_piece(opT, n, B, half):
                def emit():
                    ps = misc_psum.tile([P, QC], F32, tag="misc", name="opps")
                    for a2 in range(ET):
                        nc.tensor.matmul(
                            ps[:],
                            opT[:, a2, B, :],
                            wo_sb[:, 4 * half : 4 * half + 4, a2, :],
                            start=(a2 == 0),
                            stop=(a2 == ET - 1),
                        )
                    op_sb = op_pool.tile([P, QC], MM_DT, tag="op")
                    nc.vector.tensor_copy(op_sb[:], ps[:])
                    r0 = n * TPN + B * TPB
                    nc.sync.dma_start(
                        out=outp[r0 : r0 + TPB, half * QC : (half + 1) * QC],
                        in_=op_sb[:],
                    )
                return emit

            # ---- main pipeline (j-PAIRED software-pipelined emission) ----
            N_RUN = int(_os.environ.get("BATCHES", N))
            units = [
                (n, B, g, j)
                for n in range(N_RUN)
                for B in range(BPC)
                for g in range(2)
                for j in range(NS)
            ]
            NU = len(units)
            feed = []
            tiles_by_pair = {}
            ctx_by_batch = {}
            pv_by_phase = {}
            exps_by_unit = {}

            tiles0, groups0 = project_pair(0)
            tiles_by_pair[0] = tiles0
            for g_ in groups0:
                g_()

            def batch_start(n):
                if n % 2 == 0 and n // 2 + 1 < (N_RUN + 1) // 2:
                    nxt = project_pair(n // 2 + 1)
                    tiles_by_pair[n // 2 + 1] = nxt[0]
                    feed.extend(nxt[1])
                opT = opt_pool.tile([P, ET, BPC, TPB], MM_DT, tag="opT",
                                    name="opT")
                sums = [
                    rec_pool.tile([P, QC], F32, tag=f"sums{_i}",
                                  name=f"sums{_i}")
                    for _i in range(2)
                ]
                for _i in range(2):
                    nc.vector.memset(sums[_i][:], 1.0)
                ctx_by_batch[n] = (opT, sums)

            def make_norm_group(opT, rec, B):
                # the four 1/denom broadcasts of block (n,B) batched in one
                # feed item: pairs of row-packed concurrent matmuls (distinct
                # 32-row groups), then DVE multiplies normalize opT in place.
                def emit():
                    for half in range(2):
                        bcps = []
                        for sg_ in range(2):
                            c = B * 4 + sg_ * 2 + half
                            rp_ = 32 * (c % 4)
                            bcp = misc_psum.tile([P, QC], F32, tag="misc",
                                                 name="bcp")
                            bcps.append(bcp)
                            nc.tensor.matmul(
                                bcp[:],
                                ones_r[rp_ : rp_ + 1, :],
                                rec[rp_ : rp_ + 1, :],
                                start=True,
                                stop=True,
                                tile_position=(rp_, 0),
                            )
                        for sg_ in range(2):
                            g_ = half
                            dst = opT[sg_ * DK : (sg_ + 1) * DK,
                                      4 * g_ : 4 * g_ + 4, B, :]
                            nc.vector.tensor_mul(
                                dst,
                                dst,
                                bcps[sg_][sg_ * DK : (sg_ + 1) * DK, :]
                                .rearrange("d (a t) -> d a t", t=TPB),
                            )
                return emit

            def block_end(n, B):
                # as soon as block (n,B)'s PVs finish: reciprocal of its
                # denominators, then queue its norm + out-projection pieces.
                opT, sums = ctx_by_batch[n]
                rec = rec_pool.tile([P, QC], mybir.dt.float32r,
                                    tag=f"rec{B}", name=f"rec{B}")
                rsc = rec_pool.tile([P, QC], F32, tag=f"rsc{B}",
                                    name=f"rsc{B}")
                with nc.allow_low_precision(reason="softmax denominators"):
                    nc.vector.reciprocal_approx_fast(rsc[:], sums[B][:])
                    nc.vector.tensor_copy(rec[:], rsc[:])
                feed.append(make_norm_group(opT, rec, B))
                for half in range(2):
                    feed.append(make_outproj_piece(opT, n, B, half))

            def emit_scores_exp(k):
                n, B, g, j = units[k]
                if j == 0 and B == 0 and g == 0:
                    batch_start(n)
                q_sb, k_dup, v_sb = tiles_by_pair[n // 2]
                nn = n % 2
                if j == 0:
                    pv_by_phase[(n, B, g)] = [
                        pv_psum.tile([DK + 1, QC], F32, tag="pv",
                                     name=f"pv{_s}")
                        for _s in range(2)
                    ]
                tsl = slice(B * TPB, (B + 1) * TPB)
                sc = sc_psum.tile([P, 2, QC], F32, tag="sc", name="sc")
                exps = exp_pool.tile([P, 2 * QC], MM_DT, tag="exps",
                                     name="exps")
                exps_by_unit[k] = exps
                for sg in range(2):
                    dsl = slice(sg * DK, (sg + 1) * DK)
                    nc.tensor.matmul(
                        sc[:, sg, :],
                        k_dup[dsl, nn, j, tsl],
                        q_sb[dsl, nn, B, g, :, :],
                        start=True,
                        stop=True,
                    )
                sc_flat = sc.rearrange("p s q -> p (s q)")
                if j in DVE_JS[B]:
                    with nc.allow_low_precision(reason="schraudolph exp"):
                        nc.vector.tensor_scalar(
                            exps[:].bitcast(I16),
                            sc_flat,
                            SCHR_A,
                            SCHR_B,
                            AluOpType.mult,
                            AluOpType.add,
                        )
                else:
                    nc.scalar.activation(
                        exps[:],
                        sc_flat,
                        mybir.ActivationFunctionType.Exp,
                        scale=1.0 / np.sqrt(DK),
                    )

            def emit_pv_and_finish(k):
                n, B, g, j = units[k]
                q_sb, k_dup, v_sb = tiles_by_pair[n // 2]
                nn = n % 2
                pv = pv_by_phase[(n, B, g)]
                exps = exps_by_unit.pop(k)
                for sg in range(2):
                    nc.tensor.matmul(
                        pv[sg][:],
                        v_sb[:, nn, B, j, :],
                        exps[:, sg * QC : (sg + 1) * QC],
                        start=(j == 0),
                        stop=(j == NS - 1),
                    )
                if j == NS - 1:
                    opT, sums = ctx_by_batch[n]
                    qa = slice(4 * g, 4 * g + 4)
                    for sg in range(2):
                        c = B * 4 + sg * 2 + g
                        rp = 32 * (c % 4)
                        nc.vector.tensor_copy(
                            sums[c // 4][rp : rp + 1, :],
                            pv[sg][DK : DK + 1, :],
                        )
                        # split the two opT evictions across ScalarE and the
                        # DVE so the pv banks recycle ~2x sooner (the next
                        # phase's first PV has a WAR on them).
                        dst = opT[sg * DK : (sg + 1) * DK, qa, B, :]
                        src = pv[sg][0:DK, :].rearrange(
                            "d (a t) -> d a t", t=TPB
                        )
                        if sg == 0:
                            nc.scalar.activation(
                                dst, src, mybir.ActivationFunctionType.Copy
                            )
                        else:
                            nc.vector.tensor_copy(dst, src)
                    del pv_by_phase[(n, B, g)]
                    if g == 1:
                        block_end(n, B)

            # scores run two units ahead, emitted in adjacent PAIRS so the
            # two row-packed score matmul pairs sit back-to-back on the PE.
            emit_scores_exp(0)
            emit_scores_exp(1)
            NP2 = NU // 2
            for m in range(NP2 + 1):
                if m + 1 <= NP2 - 1:
                    emit_scores_exp(2 * m + 2)
                    emit_scores_exp(2 * m + 3)
                if feed:
                    rem = 32 - (m % 32)
                    take = (len(feed) + rem - 1) // rem
                    for _ in range(min(take, len(feed))):
                        feed.pop(0)()
                if m < NP2:
                    emit_pv_and_finish(2 * m)
                    emit_pv_and_finish(2 * m + 1)

            while feed:
                feed.pop(0)()

    nc.compile()
    return nc


_CACHED_NC = None


def get_nc():
    global _CACHED_NC
    if _CACHED_NC is None:
        _CACHED_NC = build_nc()
    return _CACHED_NC


def make_in_maps(inputs):
    x = np.ascontiguousarray(np.asarray(inputs["x"], dtype=np.float32))
    Wq = np.asarray(inputs["Wq"], dtype=np.float32)
    Wk = np.asarray(inputs["Wk"], dtype=np.float32)
    Wv = np.asarray(inputs["Wv"], dtype=np.float32)
    Wo = np.asarray(inputs["Wo"], dtype=np.float32)

    def cast(a):
        return np.ascontiguousarray(a).astype(ml_dtypes.bfloat16)

    def prew(W):
        # w_pre[p, c, a, d] = W.T[a*128+p, c*128+d]
        wT = W.T.reshape(ET, P, ET, P)
        return cast(wT.transpose(1, 2, 0, 3))

    wqT = prew(Wq)
    wkT = prew(Wk)
    wvT = prew(Wv)
    woT = prew(Wo)
    xr = x.reshape(N, L, E)

    in_maps = []
    for c in range(NC):
        xc = xr[:, 256 * c : 256 * (c + 1), :]  # [N, 256, E]
        # xp[p, n, a, t] = xc[n, t, a*128+p]
        xpc = xc.transpose(2, 0, 1).reshape(ET, P, N, TPN).transpose(1, 2, 0, 3)
        in_maps.append(
            {
                "xp": cast(xpc),
                "wqT": wqT,
                "wkT": wkT,
                "wvT": wvT,
                "woT": woT,
            }
        )
    return in_maps


def kernel(x, Wq, Wk, Wv, Wo):
    in_maps = make_in_maps({"x": x, "Wq": Wq, "Wk": Wk, "Wv": Wv, "Wo": Wo})
    res = run_bass_kernel_spmd(get_nc(), in_maps, list(range(NC)))
    out = np.empty((N, L, E), dtype=np.float32)
    for c in range(NC):
        o = res.results[c]["outp"].astype(np.float32).reshape(N, TPN, E)
        out[:, 256 * c : 256 * (c + 1), :] = o
    return out
